# revision 33
# baseline (speedup 1.0000x reference)
"""Multi-head causal attention (dense transformer block) on 8 Trainium2 cores.

Sharding: 2-way data parallel over batch x 4-way tensor parallel over heads.
Core c handles batch c//4 and heads 4*(c%4) .. 4*(c%4)+3.

Per-core pipeline (all activation layouts chosen so no on-device transposes
are needed; host pre-transposes x and the weight shards once):
  1. QT/KT [hd, t] and V [t, hd] projections from xT [d, t]. One superstep
     over all T tokens: weights are loaded exactly once (bf16 halves both
     the weight and x traffic vs the fp32 two-superstep variant).
  2. Attention per (head, q-chunk) with scores computed transposed
     (S^T [k, q]), exp without max-subtraction (scores are O(1) so exp is
     safe), causal masking via a DVE multiply with a constant triangle
     (keeps the Pool queue free for the collective), softmax denominators
     via ones-vector matmuls, AV accumulated as out^T [hd, q].
  3. Output projection final[t, e] = sum_c out^T[c, t] * woT[c, e] (partial
     sum over this core's heads), written to a ts-major bounce buffer so
     the whole rep needs only ONE ReduceScatter.
  4. One ReduceScatter per rep over the 4 cores sharing a batch; the host
     undoes the ts-major permutation when assembling the full output.

Matmul inputs are bf16 by default (ATTN_MM_DTYPE=float32r switches back to
full-rate fp32). PSUM accumulation stays fp32; softmax denominators are
reduced from an fp32 psum.
"""

import os
import sys

sys.path.insert(0, "/opt/trn_rl_repo")

import numpy as np

N_CORES = 8
B = 2
T = 2048          # sequence length
D = 2048          # model dim
P = 128           # partitions
HD = 128          # head dim
NHG = 4           # head-groups (cores per batch)
HPC = 4           # heads per core
F = HPC * HD      # 512 per-core q/k/v feature width
TC = 512          # token chunk (attention free dim)
NTC = T // TC     # 4 token chunks
ND = D // P       # 16 d-subtiles
NJ = ND // 2      # 8 packed weight tiles (2 d-subtiles each)
SCALE = float(HD) ** -0.5

_CACHE = {}


def _build(mm_dtype_name: str, reps: int = 1, with_rs: bool = True,
           skew: int = 2, den_dve: bool = True):
    import concourse.bacc as bacc
    import concourse.mybir as mybir
    import concourse.tile as tile

    dt = mybir.dt
    f32 = dt.float32
    md = getattr(dt, mm_dtype_name)  # dtype of every PE-input tile
    fdim = 512  # moving free-dim per matmul (ISA cap)

    nc = bacc.Bacc(
        "TRN2", target_bir_lowering=False, debug=False, num_devices=N_CORES
    )

    xT = nc.dram_tensor("xT", [D, T], md, kind="ExternalInput")
    # packed: [j*128+p, sub*512+f] = W^T[(2j+sub)*128+p, f]
    wqP = nc.dram_tensor("wqP", [D // 2, 2 * F], md, kind="ExternalInput")
    wkP = nc.dram_tensor("wkP", [D // 2, 2 * F], md, kind="ExternalInput")
    wvP = nc.dram_tensor("wvP", [D // 2, 2 * F], md, kind="ExternalInput")
    # packed: [(ci*2+etp)*128+p, f] = W_out^T[ci*128+p, etp*1024+f]
    woP = nc.dram_tensor("woP", [F * 2, D // 2], md, kind="ExternalInput")
    # ts-major partial sums: row ((ts*NTC+qt)*2+etp)*128+p holds
    # final-partial[qt*TC+ts*P+p, etp*1024 .. +1024]; the flat 4-way split
    # of a ReduceScatter then hands core r exactly its ts=r block.
    out = nc.dram_tensor("out", [NTC * 2 * P, D // 2], md,
                         kind="ExternalOutput")

    with nc.allow_low_precision(reason="low-precision matmul-input tiles"), \
         tile.TileContext(nc) as tc:
        with (
            tc.tile_pool(name="const", bufs=1) as const,
            tc.tile_pool(name="resident", bufs=1) as res_pool,
            tc.tile_pool(name="dram", bufs=1, space="DRAM") as dram,
        ):
            ones_stage = const.tile([P, P], f32)
            nc.vector.memset(ones_stage[:], 1.0)
            ones_col = const.tile([P, 1], md)
            nc.scalar.copy(ones_col[:], ones_stage[:, 0:1])
            ones_col_r = const.tile([P, 1], dt.float32r)
            nc.scalar.copy(ones_col_r[:], ones_stage[:, 0:1])
            ones_row = const.tile([1, P], md)
            nc.scalar.copy(ones_row[:], ones_stage[0:1, :])

            # Causal-mask constant [P, 2P]: col c keeps iff c >= P + p, so
            # cols [0, P) are all-zero (dead strip) and cols [P, 2P) hold
            # the upper-triangular keep mask.
            mask_stage = const.tile([P, 2 * P], f32)
            nc.vector.memset(mask_stage[:], 1.0)
            nc.gpsimd.affine_select(
                mask_stage[:], mask_stage[:],
                pattern=[[1, 2 * P]],
                compare_op=mybir.AluOpType.is_ge,
                fill=0.0,
                base=-P,
                channel_multiplier=-1,
            )
            ztri = const.tile([P, 2 * P], md)
            nc.scalar.copy(ztri[:], mask_stage[:])

            # ---- resident activation buffers ----
            QT = [res_pool.tile([P, T], md, name=f"QT{h}") for h in range(HPC)]
            KT = [res_pool.tile([P, T], md, name=f"KT{h}") for h in range(HPC)]
            V = [res_pool.tile([P, F], md, name=f"V{i}") for i in range(T // P)]

            # double-buffered across reps so rep r+1's bounce writes never
            # wait on rep r's ReduceScatter read
            bounce = [dram.tile([NTC * NTC * 2 * P, D // 2], md,
                                name=f"bounce{i}") for i in range(2)]
            rs_out = [dram.tile([NTC * 2 * P, D // 2], md,
                                name=f"rs_out{i}") for i in range(2)]

            for rep in range(reps):
                _build_body(nc, tc, mybir, md, f32, fdim, rep,
                            xT, wqP, wkP, wvP, woP, out,
                            ones_col, ones_col_r, ones_row, ztri, QT, KT, V,
                            bounce[rep % 2], rs_out[rep % 2], with_rs,
                            skew, den_dve)

    nc.compile()
    return nc


def _build_body(nc, tc, mybir, md, f32, fdim, rep,
                xT, wqP, wkP, wvP, woP, out,
                ones_col, ones_col_r, ones_row, ztri, QT, KT, V,
                bounce, rs_out, with_rs=True, skew=2, den_dve=False):
    # ---- phase 1: projections (single superstep over all T tokens) ----
    nth = T // fdim
    with tc.tile_pool(name=f"psum1_{rep}", bufs=1, space="PSUM") as psum1, \
         tc.tile_pool(name=f"xw_{rep}", bufs=3) as xw_pool:
        xts = []
        for di in range(ND):
            xt = xw_pool.tile(
                [P, T], md, name=f"xt_{rep}_{di}", tag="xt", bufs=ND + 2,
            )
            nc.sync.dma_start(
                xt[:], xT.ap()[di * P:(di + 1) * P, :],
            )
            xts.append(xt)
        wts = {}
        for wname, wP in (("q", wqP), ("k", wkP), ("v", wvP)):
            for j in range(NJ):
                wt = xw_pool.tile(
                    [P, 2 * F], md, name=f"w{wname}_{rep}_{j}",
                    tag="wt", bufs=8,
                )
                nc.scalar.dma_start(wt[:], wP.ap()[j * P:(j + 1) * P, :])
                wts[wname, j] = wt
        for wname, dest in (("q", QT), ("k", KT)):
            for h in range(HPC):
                for th in range(nth):
                    ps = psum1.tile(
                        [P, fdim], f32, name=f"ps_{wname}{h}{th}_{rep}",
                        tag="pq", bufs=2,
                    )
                    for j in range(NJ):
                        wt = wts[wname, j]
                        for sub in range(2):
                            di = 2 * j + sub
                            nc.tensor.matmul(
                                ps[:],
                                wt[:, sub * F + h * HD:
                                   sub * F + (h + 1) * HD],
                                xts[di][:, th * fdim:(th + 1) * fdim],
                                start=(di == 0),
                                stop=(di == ND - 1),
                            )
                    nc.any.tensor_copy(
                        dest[h][:, th * fdim:(th + 1) * fdim], ps[:]
                    )
        for ts in range(T // P):
            ps = psum1.tile(
                [P, F], f32, name=f"ps_v{ts}_{rep}", tag="pv", bufs=2,
            )
            for j in range(NJ):
                wt = wts["v", j]
                for sub in range(2):
                    di = 2 * j + sub
                    nc.tensor.matmul(
                        ps[:],
                        xts[di][:, ts * P:(ts + 1) * P],
                        wt[:, sub * F:(sub + 1) * F],
                        start=(di == 0),
                        stop=(di == ND - 1),
                    )
            nc.any.tensor_copy(V[ts][:], ps[:])

    # ---- phases 2+3 per q chunk ----
    with tc.tile_pool(name=f"psum2_{rep}", bufs=1, space="PSUM") as psum2, \
         tc.tile_pool(name=f"work_{rep}", bufs=6) as work:
        WO = []
        for ci in range(HPC):
            row = []
            for etp in range(2):
                wo = work.tile([P, D // 2], md, name=f"WO{rep}_{ci}_{etp}",
                               tag=f"WO{ci}_{etp}", bufs=1)
                nc.sync.dma_start(
                    wo[:],
                    woP.ap()[(ci * 2 + etp) * P:(ci * 2 + etp + 1) * P, :],
                )
                row.append(wo)
            WO.append(row)
        def emit_piece(pqt, ts, etp, p_outT):
            # one (ts, etp) piece of chunk pqt's output projection
            fin = work.tile(
                [P, D // 2], md, name=f"fin{rep}_{pqt}_{ts}_{etp}",
                tag="fin", bufs=3,
            )
            psf = [
                psum2.tile(
                    [P, TC], f32,
                    name=f"ps_f{rep}_{pqt}_{ts}_{etp}_{ee}",
                    tag="f", bufs=2,
                )
                for ee in range(2)
            ]
            for ci in range(HPC):
                for ee in range(2):
                    nc.tensor.matmul(
                        psf[ee][:],
                        p_outT[ci][:, ts * P:(ts + 1) * P],
                        WO[ci][etp][:, ee * TC:(ee + 1) * TC],
                        start=(ci == 0),
                        stop=(ci == HPC - 1),
                    )
            for ee in range(2):
                nc.any.tensor_copy(
                    fin[:, ee * TC:(ee + 1) * TC], psf[ee][:]
                )
            drow = ((ts * NTC + pqt) * 2 + etp) * P
            nc.sync.dma_start(bounce[drow:drow + P, :], fin[:])

        prev_outT = None
        for qt in range(NTC):
            outT = {}
            # previous chunk's projection pieces, interleaved into this
            # chunk's attention emission so the PE queue has dense filler
            # during the exp/DVE latency windows
            pieces = ([(ts, etp) for ts in range(TC // P)
                       for etp in range(2)] if prev_outT else [])
            n_k = (qt + 1) * (TC // P)  # causal: k-subtiles needed
            diag0 = qt * (TC // P)
            korder = list(range(diag0, n_k)) + list(range(diag0))
            SKEW = skew
            for hp in (0, 2):  # head pairs, emission interleaved
                heads = (hp, hp + 1)
                ps_out = {
                    h: psum2.tile(
                        [P, TC], f32, name=f"ps_out{rep}_{qt}_{h}",
                        tag="out", bufs=2,
                    )
                    for h in heads
                }
                ps_den = {}
                accs = {}
                if den_dve:
                    # bf16 acc keeps both tensor_add operands 2-byte packed
                    # so the DVE runs its 2x/4x mode and stays off the
                    # attention inner loop's critical path; the rounding
                    # random-walk over <=16 adds costs ~4e-3 on den.
                    accs = {
                        h: work.tile(
                            [P, TC], md,
                            name=f"acc{rep}_{qt}_{h}", tag="acc", bufs=6,
                        )
                        for h in heads
                    }
                else:
                    ps_den = {
                        h: psum2.tile(
                            [1, TC], f32, name=f"ps_den{rep}_{qt}_{h}",
                            tag="aux", bufs=2,
                        )
                        for h in heads
                    }
                pts = {}
                # For diagonal tiles only columns q >= 128*dj are live:
                # S/exp/AV/den all operate on that sub-rectangle, and the
                # causal mask shrinks to one 128x128 triangle block. korder
                # starts at dj=0 (full width), so the start=True matmuls
                # initialize every psum column's has_written bit. bf16
                # matmuls run full-rate at any width, so widths are exact.
                def live0(kt):
                    dj = kt - diag0
                    return min(max(0, dj) * P, TC - P)
                for step in range(n_k + SKEW):
                    if step < n_k:
                        kt = korder[step]
                        c0 = live0(kt)
                        for h in heads:
                            ps_st = psum2.tile(
                                [P, TC], f32,
                                name=f"ps_st{rep}_{qt}_{h}_{kt}",
                                tag="st", bufs=2,
                            )
                            nc.tensor.matmul(
                                ps_st[:, c0:],
                                KT[h][:, kt * P:(kt + 1) * P],
                                QT[h][:, qt * TC + c0:(qt + 1) * TC],
                                start=True,
                                stop=True,
                            )
                            pt = work.tile(
                                [P, TC], md, name=f"pt{rep}_{qt}_{h}_{kt}",
                                tag="pt", bufs=8,
                            )
                            nc.scalar.activation(
                                pt[:, c0:], ps_st[:, c0:],
                                mybir.ActivationFunctionType.Exp,
                                scale=SCALE,
                            )
                            dj = kt - diag0
                            if dj >= 0:
                                # mask [c0, (dj+1)*128): the dead strip plus
                                # the triangle block, via a DVE multiply
                                # (keeps Pool free for the collective)
                                me = (dj + 1) * P
                                nc.vector.tensor_mul(
                                    pt[:, c0:me], pt[:, c0:me],
                                    ztri[:, 2 * P - (me - c0):],
                                )
                            if den_dve:
                                if step == 0:
                                    nc.scalar.copy(accs[h][:], pt[:])
                                else:
                                    nc.vector.tensor_add(
                                        accs[h][:, c0:], accs[h][:, c0:],
                                        pt[:, c0:],
                                    )
                            pts[h, kt] = pt
                    if step >= SKEW:
                        idx = step - SKEW
                        k = korder[idx]
                        c0 = live0(k)
                        for h in heads:
                            if not den_dve:
                                nc.tensor.matmul(
                                    ps_den[h][:, c0:],
                                    ones_col[:],
                                    pts[h, k][:, c0:],
                                    start=(idx == 0),
                                    stop=(idx == n_k - 1),
                                )
                            nc.tensor.matmul(
                                ps_out[h][:, c0:],
                                V[k][:, h * HD:(h + 1) * HD],
                                pts[h, k][:, c0:],
                                start=(idx == 0),
                                stop=(idx == n_k - 1),
                            )
                    if pieces and step % 2 == 1:
                        ts_, etp_ = pieces.pop(0)
                        emit_piece(qt - 1, ts_, etp_, prev_outT)
                for h in heads:
                    if den_dve:
                        ps_den[h] = psum2.tile(
                            [1, TC], f32, name=f"ps_den{rep}_{qt}_{h}",
                            tag="aux", bufs=2,
                        )
                        nc.tensor.matmul(
                            ps_den[h][:], ones_col[:], accs[h][:],
                            start=True, stop=True,
                        )
                    den = work.tile([1, TC], md, name=f"den{rep}_{qt}_{h}",
                                    tag="den", bufs=2)
                    nc.vector.reciprocal(den[:], ps_den[h][:])
                    ps_bc = psum2.tile(
                        [P, TC], f32, name=f"ps_bc{rep}_{qt}_{h}", tag="aux",
                        bufs=2,
                    )
                    nc.tensor.matmul(
                        ps_bc[:], ones_row[:], den[:],
                        start=True, stop=True,
                    )
                    bc = work.tile([P, TC], f32, name=f"bc{rep}_{qt}_{h}",
                                   tag="bc", bufs=2)
                    nc.any.tensor_copy(bc[:], ps_bc[:])
                    # bufs=8: the previous chunk's four outT tiles stay live
                    # while this chunk produces its four
                    ot = work.tile([P, TC], md, name=f"outT{rep}_{qt}_{h}",
                                   tag="outT", bufs=8)
                    nc.vector.tensor_mul(ot[:], ps_out[h][:], bc[:])
                    outT[h] = ot

            # flush any previous-chunk pieces not absorbed by the steps
            for ts_, etp_ in pieces:
                emit_piece(qt - 1, ts_, etp_, prev_outT)
            prev_outT = outT
        # last chunk's projection has no following attention to hide in
        for ts_ in range(TC // P):
            for etp_ in range(2):
                emit_piece(NTC - 1, ts_, etp_, prev_outT)
        # ---- phase 4: one ReduceScatter per rep ----
        if with_rs:
            nc.gpsimd.collective_compute(
                "ReduceScatter",
                mybir.AluOpType.add,
                replica_groups=[[0, 1, 2, 3], [4, 5, 6, 7]],
                ins=[bounce.opt()],
                outs=[rs_out.opt()],
            )
            nc.sync.dma_start(out.ap()[:, :], rs_out[:])
        else:
            nc.sync.dma_start(out.ap()[:, :],
                              bounce[0:NTC * 2 * P, :])


def _get_nc():
    name = os.environ.get("ATTN_MM_DTYPE", "bfloat16")
    reps = int(os.environ.get("ATTN_REPS", "1"))
    key = (name, reps)
    if key not in _CACHE:
        _CACHE[key] = _build(name, reps)
    return _CACHE[key]


last_exec_time_ns = None


def _np_dtype(name):
    if name == "bfloat16":
        import ml_dtypes

        return np.dtype(ml_dtypes.bfloat16)
    return np.dtype(np.float32)


def _pack_w(wT, npdt):
    # [2048, 512] -> [1024, 1024]: packed[j*128+p, sub*512+f] =
    # wT[(2j+sub)*128+p, f]
    return np.ascontiguousarray(
        wT.reshape(NJ, 2, P, F).swapaxes(1, 2).reshape(D // 2, 2 * F)
    ).astype(npdt)


def make_in_maps(x, w_qkv, w_out, mm_dtype_name=None):
    if mm_dtype_name is None:
        mm_dtype_name = os.environ.get("ATTN_MM_DTYPE", "bfloat16")
    npdt = _np_dtype(mm_dtype_name)
    x = np.asarray(x, dtype=np.float32)
    w_qkv = np.asarray(w_qkv, dtype=np.float32)
    w_out = np.asarray(w_out, dtype=np.float32)
    xTs = [np.ascontiguousarray(x[b].T).astype(npdt) for b in range(B)]
    in_maps = []
    for c in range(N_CORES):
        b, hg = divmod(c, NHG)
        sl = slice(hg * F, (hg + 1) * F)
        woT = w_out[:, sl].T  # [512, 2048]
        woPk = np.ascontiguousarray(
            woT.reshape(HPC, P, 2, D // 2).transpose(0, 2, 1, 3)
            .reshape(F * 2, D // 2)
        ).astype(npdt)
        in_maps.append({
            "xT": xTs[b],
            "wqP": _pack_w(w_qkv[0 * D:1 * D][sl].T, npdt),
            "wkP": _pack_w(w_qkv[1 * D:2 * D][sl].T, npdt),
            "wvP": _pack_w(w_qkv[2 * D:3 * D][sl].T, npdt),
            "woP": woPk,
        })
    return in_maps


def kernel(x, w_qkv, w_out):
    import time

    from concourse import bass_utils

    global last_exec_time_ns
    nc = _get_nc()
    in_maps = make_in_maps(x, w_qkv, w_out)

    trace = bool(int(os.environ.get("ATTN_TRACE", "0")))
    res = None
    last_err = None
    for attempt in range(3):
        try:
            res = bass_utils.run_bass_kernel_spmd(
                nc, in_maps, core_ids=list(range(N_CORES)), trace=trace
            )
            break
        except Exception as e:  # transient axon mesh desyncs
            last_err = e
            time.sleep(10 * (attempt + 1))
    if res is None:
        raise last_err
    last_exec_time_ns = res.exec_time_ns

    outs = [np.asarray(res.results[c]["out"], dtype=np.float32)
            for c in range(N_CORES)]
    # core r of a batch group holds rows [qt, etp, p] -> full row
    # qt*TC + r*P + p, columns etp*1024 .. +1024
    full = []
    for b in range(B):
        fb = np.empty((T, D), np.float32)
        for r in range(NHG):
            arr = outs[b * NHG + r].reshape(NTC, 2, P, D // 2)
            for qt in range(NTC):
                rows = slice(qt * TC + r * P, qt * TC + (r + 1) * P)
                fb[rows, 0:D // 2] = arr[qt, 0]
                fb[rows, D // 2:D] = arr[qt, 1]
        full.append(fb)
    return np.stack(full)
